# revision 15
# baseline (speedup 1.0000x reference)
"""Cluster-attention (moe_routing) Bass kernel for TRN2, 8 NeuronCores.

Math (see reference):
    q    = mean_n x[b,n,:]                       [B,C]
    attn = softmax(l2norm(q) @ l2norm_col(centers))   [B,K]
    tm   = einsum('bk,knm->bnm', attn, tran_ms)  [B,N,N]
    y    = einsum('bnm,bmc->bnc', tm, x)
    out  = y @ proj_w.T + proj_b

Sharding: output rows n (N=512) are split across the 8 cores (64 rows
per core).  Each core receives only its slice tran_ms[:, n0:n0+64, :]
(so the dominant 67MB tensor is read once system-wide), the full x
(needed for the m-contraction and the mean-pool), and the small
centers/proj tensors.  No collectives: each core independently computes
out[:, n0:n0+64, :] and the host concatenates.

Per-core dataflow (all matmuls keyed to avoid any big transposes):
  q:      ones[128,1].T @ x-tiles             -> q rows (PSUM), scatter to Q[16,384]
  attn:   DVE/ACT softmax chain on 16 partitions; PE transposes for the
          [c,b]-layout logits operand and for attnT[64,16].
  step5:  out[m128,b16] = T-tile[k64, m128].T @ attnT[k64, b16]
          -> tmT[m, b, n] with m on partitions (what step6 needs).
  step6:  yT[c128,n64]  = x-tile[m128, c128].T @ tmT[m128, (b,) n64], acc over 4 m-chunks.
  step7:  out[n64,c384] = yT[ci128, n64].T @ projT[ci128, co384], acc over 3 ci-chunks,
          + bias (broadcast over partitions via a K=1 PE matmul), DMA out.
"""

import sys

import numpy as np

sys.path.insert(0, "/opt/trn_rl_repo")

import concourse.bacc as bacc  # noqa: E402
import concourse.mybir as mybir  # noqa: E402
import concourse.tile as tile  # noqa: E402
from concourse import masks  # noqa: E402
from concourse.alu_op_type import AluOpType  # noqa: E402

B, N, C, K = 16, 512, 384, 64
NCORES = 8
NL = N // NCORES  # 64 output rows per core
MC = N // 128  # 4 m-chunks (contraction over m in step6)
CB = C // 128  # 3 c-blocks
F32 = mybir.dt.float32
F32R = mybir.dt.float32r
AF = mybir.ActivationFunctionType
AX = mybir.AxisListType

T_CHUNK = 4  # n-rows of tran_ms streamed per DMA chunk
N_TCH = NL // T_CHUNK


def _emit(tc, x_d, cen_d, pw_d, pb_d, t_d, out_d, it=0, f32r_proj=True):
    from contextlib import ExitStack

    nc = tc.nc
    ctx = ExitStack()
    sing = ctx.enter_context(tc.tile_pool(name="sing", bufs=1))
    tpool = ctx.enter_context(tc.tile_pool(name="tstream", bufs=4))
    opool = ctx.enter_context(tc.tile_pool(name="ostage", bufs=4))
    ps_s = ctx.enter_context(tc.tile_pool(name="ps_s", bufs=2, space="PSUM"))
    ps5 = ctx.enter_context(tc.tile_pool(name="ps5", bufs=2, space="PSUM"))
    ps6 = ctx.enter_context(tc.tile_pool(name="ps6", bufs=2, space="PSUM"))
    ps7 = ctx.enter_context(tc.tile_pool(name="ps7", bufs=2, space="PSUM"))

    # --- persistent SBUF tensors ---
    x_sb = sing.tile([128, B * MC * C], F32)  # x[b] m-chunk tiles, (b, mc) major
    cen_sb = sing.tile([128, CB * K], F32)
    pw_sb = sing.tile([128, CB * C], F32)  # proj_w natural [co_chunk, ci]
    pjT_sb = sing.tile([128, CB * C], F32)  # proj_w.T       [ci_chunk, co]
    bias_sb = sing.tile([1, C], F32)
    bias64 = sing.tile([NL, C], F32)
    ones128 = sing.tile([128, 1], F32)
    ones16 = sing.tile([1, B], F32)
    ones64 = sing.tile([1, NL], F32)
    ident = sing.tile([128, 128], F32)
    q_sb = sing.tile([B, C], F32)
    qsq = sing.tile([B, C], F32)
    qss = sing.tile([B, 1], F32)
    qinv = sing.tile([B, 1], F32)
    qn = sing.tile([B, C], F32)
    qnT = sing.tile([128, CB * B], F32)
    csq = sing.tile([128, CB * K], F32)
    sinv = sing.tile([1, K], F32)
    sb_bc = sing.tile([B, K], F32)
    logit = sing.tile([B, K], F32)
    mx = sing.tile([B, 1], F32)
    esh = sing.tile([B, K], F32)
    ex = sing.tile([B, K], F32)
    se = sing.tile([B, 1], F32)
    rinv = sing.tile([B, 1], F32)
    attn = sing.tile([B, K], F32)
    attnT = sing.tile([K, B], F32)
    tmT = sing.tile([128, MC * B * NL], F32)
    yT = sing.tile([128, CB * B * NL], F32)

    tmT_v = tmT[:].rearrange("p (mc b n) -> p mc b n", mc=MC, b=B)
    yT_v = yT[:].rearrange("p (cb b n) -> p cb b n", cb=CB, b=B)

    nc.gpsimd.memset(ones128[:], 1.0)
    nc.gpsimd.memset(ones16[:], 1.0)
    nc.gpsimd.memset(ones64[:], 1.0)
    masks.make_identity(nc, ident[:])

    # --- loads ---
    for b in range(B):
        for mc in range(MC):
            nc.sync.dma_start(
                out=x_sb[:, (b * MC + mc) * C : (b * MC + mc + 1) * C],
                in_=x_d[b, mc * 128 : (mc + 1) * 128, :],
            )
    for i in range(CB):
        nc.sync.dma_start(
            out=cen_sb[:, i * K : (i + 1) * K], in_=cen_d[i * 128 : (i + 1) * 128, :]
        )
        nc.sync.dma_start(
            out=pw_sb[:, i * C : (i + 1) * C], in_=pw_d[i * 128 : (i + 1) * 128, :]
        )
    nc.sync.dma_start(out=bias_sb[:], in_=pb_d.unsqueeze(0))

    # --- q = mean_n x ---
    # DVE pre-reduce over the 4 m-chunks (innermost axis of the strided
    # view), then one cross-partition PE matmul per b.
    q_dram = nc.dram_tensor(f"q_scratch_{it}", [B, C], F32).ap()
    with tc.tile_pool(name="xsum", bufs=3) as xsp, tc.tile_pool(name="qrow", bufs=2) as qrp:
        for b in range(B):
            xs = xsp.tile([128, C], F32, tag="xs")
            xv = x_sb[:, b * MC * C : (b + 1) * MC * C].rearrange(
                "p (mc c) -> p c mc", mc=MC
            )
            nc.vector.reduce_sum(xs[:], xv, axis=AX.X)
            ps = ps_s.tile([1, C], F32, tag="s")
            nc.tensor.matmul(ps[:], ones128[:], xs[:])
            qr = qrp.tile([1, C], F32, tag="qr")
            nc.scalar.mul(qr[:], ps[:], 1.0 / N)
            nc.sync.dma_start(out=q_dram[b].unsqueeze(0), in_=qr[:])
    # gather rows back as [B, C] across partitions
    nc.sync.dma_start(out=q_sb[:], in_=q_dram[:])

    # --- qn = l2norm(q, axis=-1) ---
    nc.scalar.activation(qsq[:], q_sb[:], AF.Square, accum_out=qss[:])
    nc.scalar.activation(qss[:], qss[:], AF.Sqrt)
    nc.vector.reciprocal(qinv[:], qss[:])
    nc.vector.tensor_scalar_mul(qn[:], q_sb[:], qinv[:])
    for i in range(CB):
        pst = ps_s.tile([128, B], F32, tag="s")
        nc.tensor.transpose(pst[:], qn[:, i * 128 : (i + 1) * 128], ident[:B, :B])
        nc.vector.tensor_copy(qnT[:, i * B : (i + 1) * B], pst[:])

    # --- column norms of centers ---
    for i in range(CB):
        nc.scalar.activation(
            csq[:, i * K : (i + 1) * K], cen_sb[:, i * K : (i + 1) * K], AF.Square
        )
    psn = ps_s.tile([1, K], F32, tag="s")
    for i in range(CB):
        nc.tensor.matmul(
            psn[:],
            ones128[:],
            csq[:, i * K : (i + 1) * K],
            start=(i == 0),
            stop=(i == CB - 1),
        )
    snrm = sing.tile([1, K], F32)
    nc.scalar.activation(snrm[:], psn[:], AF.Sqrt)
    nc.vector.reciprocal(sinv[:], snrm[:])
    psb = ps_s.tile([B, K], F32, tag="s")
    nc.tensor.matmul(psb[:], ones16[:], sinv[:])  # broadcast over partitions
    nc.vector.tensor_copy(sb_bc[:], psb[:])

    # --- logits = (qn @ centers) * colnorm_inv ; softmax ---
    psl = ps_s.tile([B, K], F32, tag="s")
    for i in range(CB):
        nc.tensor.matmul(
            psl[:],
            qnT[:, i * B : (i + 1) * B],
            cen_sb[:, i * K : (i + 1) * K],
            start=(i == 0),
            stop=(i == CB - 1),
        )
    nc.vector.tensor_mul(logit[:], psl[:], sb_bc[:])
    nc.vector.reduce_max(mx[:], logit[:], axis=AX.X)
    nc.vector.tensor_scalar(esh[:], logit[:], mx[:], None, op0=AluOpType.subtract)
    nc.scalar.activation(ex[:], esh[:], AF.Exp, accum_out=se[:])
    nc.vector.reciprocal(rinv[:], se[:])
    nc.vector.tensor_scalar_mul(attn[:], ex[:], rinv[:])
    psa = ps_s.tile([K, B], F32, tag="s")
    nc.tensor.transpose(psa[:], attn[:], ident[:B, :B])
    nc.vector.tensor_copy(attnT[:], psa[:])

    # --- proj_w.T tiles (PE transpose, one-time) ---
    for i in range(CB):  # co chunk
        for j in range(CB):  # ci chunk
            pst = ps_s.tile([128, 128], F32, tag="s")
            nc.tensor.transpose(
                pst[:], pw_sb[:, i * C + j * 128 : i * C + (j + 1) * 128], ident[:]
            )
            pjT_dst = pjT_sb[:, j * C + i * 128 : j * C + (i + 1) * 128]
            if f32r_proj:
                pjT_dst = pjT_dst.bitcast(F32R)
            nc.vector.tensor_copy(pjT_dst, pst[:])

    # --- bias broadcast over the 64 n-partitions ---
    psbb = ps7.tile([NL, C], F32, tag="p7")
    nc.tensor.matmul(psbb[:], ones64[:], bias_sb[:])
    nc.vector.tensor_copy(bias64[:], psbb[:])

    # --- step5: tmT[m, b, n] = sum_k T[k, n, m] * attn[b, k] ---
    for ch in range(N_TCH):
        tch = tpool.tile([K, T_CHUNK, N], F32, tag="t")
        nc.sync.dma_start(out=tch[:], in_=t_d[:, ch * T_CHUNK : (ch + 1) * T_CHUNK, :])
        for nr in range(T_CHUNK):
            n0 = ch * T_CHUNK + nr
            for mc in range(MC):
                p5 = ps5.tile([128, B], F32, tag="p5")
                nc.tensor.matmul(
                    p5[:], tch[:, nr, mc * 128 : (mc + 1) * 128], attnT[:]
                )
                nc.vector.tensor_copy(tmT_v[:, mc, :, n0], p5[:])

    # --- step6: yT[c, n] per b = sum_m x[b][m, c] * tmT[m, b, n] ---
    for b in range(B):
        for cb in range(CB):
            p6 = ps6.tile([128, NL], F32, tag="p6")
            for mc in range(MC):
                nc.tensor.matmul(
                    p6[:],
                    x_sb[
                        :,
                        (b * MC + mc) * C + cb * 128 : (b * MC + mc) * C + (cb + 1) * 128,
                    ],
                    tmT_v[:, mc, b, :],
                    start=(mc == 0),
                    stop=(mc == MC - 1),
                )
            yT_dst = yT_v[:, cb, b, :]
            if f32r_proj:
                yT_dst = yT_dst.bitcast(F32R)
            nc.vector.tensor_copy(yT_dst, p6[:])

    # --- step7: out[n, co] = sum_ci yT[ci, n] * projT[ci, co] + bias ---
    for b in range(B):
        p7 = ps7.tile([NL, C], F32, tag="p7")
        for cb in range(CB):
            lhs7 = yT_v[:, cb, b, :]
            rhs7 = pjT_sb[:, cb * C : (cb + 1) * C]
            if f32r_proj:
                lhs7 = lhs7.bitcast(F32R)
                rhs7 = rhs7.bitcast(F32R)
            nc.tensor.matmul(
                p7[:], lhs7, rhs7, start=(cb == 0), stop=(cb == CB - 1)
            )
        ot = opool.tile([NL, C], F32, tag="o")
        nc.vector.tensor_add(ot[:], p7[:], bias64[:])
        nc.sync.dma_start(out=out_d[b], in_=ot[:])

    ctx.close()


_CACHE = {}


def build_nc(reps=1, f32r_proj=True):
    key = (reps, f32r_proj)
    if key in _CACHE:
        return _CACHE[key]
    nc = bacc.Bacc(
        "TRN2",
        target_bir_lowering=False,
        debug=False,
        enable_asserts=False,
        num_devices=NCORES,
    )
    x_d = nc.dram_tensor("x", [B, N, C], F32, kind="ExternalInput").ap()
    cen_d = nc.dram_tensor("centers", [C, K], F32, kind="ExternalInput").ap()
    pw_d = nc.dram_tensor("proj_w", [C, C], F32, kind="ExternalInput").ap()
    pb_d = nc.dram_tensor("proj_b", [C], F32, kind="ExternalInput").ap()
    t_d = nc.dram_tensor("t_loc", [K, NL, N], F32, kind="ExternalInput").ap()
    out_d = nc.dram_tensor("out_loc", [B, NL, C], F32, kind="ExternalOutput").ap()
    with tile.TileContext(nc) as tc:
        for it in range(reps):
            _emit(tc, x_d, cen_d, pw_d, pb_d, t_d, out_d, it=it, f32r_proj=f32r_proj)
    nc.compile()
    _CACHE[key] = nc
    return nc


def make_in_maps(x, centers, tran_ms, proj_w, proj_b):
    x = np.ascontiguousarray(x, dtype=np.float32)
    centers = np.ascontiguousarray(centers, dtype=np.float32)
    proj_w = np.ascontiguousarray(proj_w, dtype=np.float32)
    proj_b = np.ascontiguousarray(proj_b, dtype=np.float32)
    in_maps = []
    for core in range(NCORES):
        t_loc = np.ascontiguousarray(
            tran_ms[:, core * NL : (core + 1) * NL, :], dtype=np.float32
        )
        in_maps.append(
            {
                "x": x,
                "centers": centers,
                "proj_w": proj_w,
                "proj_b": proj_b,
                "t_loc": t_loc,
            }
        )
    return in_maps


def kernel(**inputs):
    from concourse.bass_utils import run_bass_kernel_spmd

    nc = build_nc()
    in_maps = make_in_maps(
        inputs["x"],
        inputs["centers"],
        inputs["tran_ms"],
        inputs["proj_w"],
        inputs["proj_b"],
    )
    res = run_bass_kernel_spmd(nc, in_maps, list(range(NCORES)))
    out = np.concatenate([res.results[c]["out_loc"] for c in range(NCORES)], axis=1)
    return out.astype(np.float32)


# revision 26
# speedup vs baseline: 777.3002x; 777.3002x over previous
"""Cluster-attention (moe_routing) Bass kernel for TRN2, 8 NeuronCores.

Math (see reference):
    q    = mean_n x[b,n,:]                       [B,C]
    attn = softmax(l2norm(q) @ l2norm_col(centers))   [B,K]
    tm   = einsum('bk,knm->bnm', attn, tran_ms)  [B,N,N]
    y    = einsum('bnm,bmc->bnc', tm, x)
    out  = y @ proj_w.T + proj_b

Sharding: output rows n (N=512) are split across the 8 cores (64 rows
per core).  Each core receives only its slice tran_ms[:, n0:n0+64, :]
(so the dominant 67MB tensor is read once system-wide), the full x
(needed for the m-contraction and the mean-pool), and the small
centers/proj tensors.  No collectives: each core independently computes
out[:, n0:n0+64, :] and the host concatenates.

Per-core dataflow (all matmuls keyed to avoid any big transposes):
  q:      DVE pre-reduce over m-chunks + ones[128,1].T @ xsum -> q rows,
          gathered to Q[16,384] through a DRAM scratch.
  attn:   DVE/ACT softmax chain on 16 partitions; PE transposes for the
          [c,b]-layout logits operand and for attnT[64,16].
  step5:  out[m128,(mc,)b16] = T-tile[k64, m128].T @ attnT[k64, b16]
          (4 m-chunks share one PSUM bank / one batched copy)
          -> tmT[m, b, n] with m on partitions (what step6 needs).
  step6:  yT[c128,n64]  = x-tile[m128, c128].T @ tmT[m128, b, n64], acc over 4 m-chunks.
  step7:  out[n64,c384] = yT[ci128, n64].T @ projT[ci128, co384], acc over 3 ci-chunks,
          + bias (broadcast over partitions via a K=1 PE matmul), DMA out.

Precision modes (PE fp32 matmul is 4 cyc/row and its weight load is slow;
f32r streams at full rate for moving free-dim >= 256 and rounds operands
to ~13-bit mantissa; bf16 is fastest, ~3e-3):
  "fp32"  : everything fp32                                 (~1e-6)
  "f32rp" : fp32 compute, f32r projection (step7) only      (~1.2e-4)
  "f32r"  : T/x washed to f32r by DVE/POOL, all-f32r matmuls (~2e-4)
  "bf16"  : T/x/attn/tm in bf16, f32r projection            (~3e-3)
"""

import sys

import numpy as np

sys.path.insert(0, "/opt/trn_rl_repo")

import concourse.bacc as bacc  # noqa: E402
import concourse.mybir as mybir  # noqa: E402
import concourse.tile as tile  # noqa: E402
from concourse import masks  # noqa: E402
from concourse.alu_op_type import AluOpType  # noqa: E402

B, N, C, K = 16, 512, 384, 64
NCORES = 8
NL = N // NCORES  # 64 output rows per core
MC = N // 128  # 4 m-chunks (contraction over m in step6)
CB = C // 128  # 3 c-blocks
F32 = mybir.dt.float32
F32R = mybir.dt.float32r
BF16 = mybir.dt.bfloat16
AF = mybir.ActivationFunctionType
AX = mybir.AxisListType

T_CHUNK = 4  # n-rows of tran_ms streamed per DMA chunk
N_TCH = NL // T_CHUNK


def _emit(tc, x_d, cen_d, pw_d, pb_d, t_d, out_d, it=0, mode="f32rp"):
    from contextlib import ExitStack

    assert mode in ("fp32", "f32rp", "f32r", "bf16")
    wash = mode == "f32r"
    bf = mode == "bf16"
    DT = BF16 if bf else F32

    def mm5(ap):  # step5/6 matmul operand cast
        return ap.bitcast(F32R) if wash else ap

    proj_f32r = mode != "fp32"

    def mm7(ap):
        return ap.bitcast(F32R) if proj_f32r else ap

    nc = tc.nc
    ctx = ExitStack()
    sing = ctx.enter_context(tc.tile_pool(name="sing", bufs=1))
    tpool = ctx.enter_context(tc.tile_pool(name="tstream", bufs=2 if wash else 3))
    opool = ctx.enter_context(tc.tile_pool(name="ostage", bufs=4))
    ps_s = ctx.enter_context(tc.tile_pool(name="ps_s", bufs=1, space="PSUM"))
    ps5 = ctx.enter_context(tc.tile_pool(name="ps5", bufs=3, space="PSUM"))
    ps6 = ctx.enter_context(tc.tile_pool(name="ps6", bufs=2, space="PSUM"))
    ps7 = ctx.enter_context(tc.tile_pool(name="ps7", bufs=2, space="PSUM"))
    if wash:
        xst_p = ctx.enter_context(tc.tile_pool(name="xstage", bufs=8))
        trp = ctx.enter_context(tc.tile_pool(name="twash", bufs=2))

    # --- persistent SBUF tensors ---
    x_sb = sing.tile([128, B * MC * C], DT)  # x[b] m-chunk tiles, (b, mc) major
    cen_sb = sing.tile([128, CB * K], F32)
    pw_sb = sing.tile([128, CB * C], F32)  # proj_w natural [co_chunk, ci]
    pjT_sb = sing.tile([128, CB * C], F32)  # proj_w.T       [ci_chunk, co]
    bias_sb = sing.tile([1, C], F32)
    bias64 = sing.tile([NL, C], F32)
    ones128 = sing.tile([128, 1], F32)
    ones16 = sing.tile([1, B], F32)
    ones64 = sing.tile([1, NL], F32)
    ident = sing.tile([128, 128], F32)
    q_sb = sing.tile([B, C], F32)
    qsq = sing.tile([B, C], F32)
    qss = sing.tile([B, 1], F32)
    qinv = sing.tile([B, 1], F32)
    qn = sing.tile([B, C], F32)
    qnT = sing.tile([128, CB * B], F32)
    csq = sing.tile([128, CB * K], F32)
    snrm = sing.tile([1, K], F32)
    sinv = sing.tile([1, K], F32)
    sb_bc = sing.tile([B, K], F32)
    logit = sing.tile([B, K], F32)
    mx = sing.tile([B, 1], F32)
    esh = sing.tile([B, K], F32)
    ex = sing.tile([B, K], F32)
    se = sing.tile([B, 1], F32)
    rinv = sing.tile([B, 1], F32)
    attn = sing.tile([B, K], F32)
    attnT = sing.tile([K, B], DT)
    tmT = sing.tile([128, MC * B * NL], DT)
    yT = sing.tile([128, CB * B * NL], F32)

    tmT_v = tmT[:].rearrange("p (mc b n) -> p mc b n", mc=MC, b=B)
    yT_v = yT[:].rearrange("p (cb b n) -> p cb b n", cb=CB, b=B)

    nc.gpsimd.memset(ones128[:], 1.0)
    nc.gpsimd.memset(ones16[:], 1.0)
    nc.gpsimd.memset(ones64[:], 1.0)
    masks.make_identity(nc, ident[:])

    # --- loads ---
    # bf16: gpsimd DMA casts fp32->bf16 in flight.
    # f32r: fp32 DMA to staging, POOL copy rounds into x_sb as f32r.
    for b in range(B):
        for mc in range(MC):
            dst = x_sb[:, (b * MC + mc) * C : (b * MC + mc + 1) * C]
            src = x_d[b, mc * 128 : (mc + 1) * 128, :]
            if wash:
                xst = xst_p.tile([128, C], F32, tag="xst")
                nc.sync.dma_start(out=xst[:], in_=src)
                nc.gpsimd.tensor_copy(dst.bitcast(F32R), xst[:])
            elif bf:
                nc.gpsimd.dma_start(out=dst, in_=src)
            else:
                nc.sync.dma_start(out=dst, in_=src)
    for i in range(CB):
        nc.sync.dma_start(
            out=cen_sb[:, i * K : (i + 1) * K], in_=cen_d[i * 128 : (i + 1) * 128, :]
        )
        nc.sync.dma_start(
            out=pw_sb[:, i * C : (i + 1) * C], in_=pw_d[i * 128 : (i + 1) * 128, :]
        )
    nc.sync.dma_start(out=bias_sb[:], in_=pb_d.unsqueeze(0))

    # --- q = mean_n x ---
    # DVE pre-reduce over the 4 m-chunks (innermost axis of the strided
    # view), then one cross-partition PE matmul per b.
    q_dram = nc.dram_tensor(f"q_scratch_{it}", [B, C], F32).ap()
    with tc.tile_pool(name="xsum", bufs=3) as xsp, tc.tile_pool(
        name="qrow", bufs=2
    ) as qrp:
        for b in range(B):
            xs = xsp.tile([128, C], F32, tag="xs")
            xv = x_sb[:, b * MC * C : (b + 1) * MC * C].rearrange(
                "p (mc c) -> p c mc", mc=MC
            )
            nc.vector.reduce_sum(xs[:], xv, axis=AX.X)
            ps = ps_s.tile([1, C], F32, tag="s")
            nc.tensor.matmul(ps[:], ones128[:], xs[:])
            qr = qrp.tile([1, C], F32, tag="qr")
            nc.scalar.mul(qr[:], ps[:], 1.0 / N)
            nc.sync.dma_start(out=q_dram[b].unsqueeze(0), in_=qr[:])
    # gather rows back as [B, C] across partitions
    nc.sync.dma_start(out=q_sb[:], in_=q_dram[:])

    # --- qn = l2norm(q, axis=-1) ---
    nc.scalar.activation(qsq[:], q_sb[:], AF.Square, accum_out=qss[:])
    nc.scalar.activation(qss[:], qss[:], AF.Sqrt)
    nc.vector.reciprocal(qinv[:], qss[:])
    nc.vector.tensor_scalar_mul(qn[:], q_sb[:], qinv[:])
    for i in range(CB):
        pst = ps_s.tile([128, B], F32, tag="s")
        nc.tensor.transpose(pst[:], qn[:, i * 128 : (i + 1) * 128], ident[:B, :B])
        nc.vector.tensor_copy(qnT[:, i * B : (i + 1) * B], pst[:])

    # --- column norms of centers ---
    for i in range(CB):
        nc.scalar.activation(
            csq[:, i * K : (i + 1) * K], cen_sb[:, i * K : (i + 1) * K], AF.Square
        )
    psn = ps_s.tile([1, K], F32, tag="s")
    for i in range(CB):
        nc.tensor.matmul(
            psn[:],
            ones128[:],
            csq[:, i * K : (i + 1) * K],
            start=(i == 0),
            stop=(i == CB - 1),
        )
    nc.scalar.activation(snrm[:], psn[:], AF.Sqrt)
    nc.vector.reciprocal(sinv[:], snrm[:])
    psb = ps_s.tile([B, K], F32, tag="s")
    nc.tensor.matmul(psb[:], ones16[:], sinv[:])  # broadcast over partitions
    nc.vector.tensor_copy(sb_bc[:], psb[:])

    # --- logits = (qn @ centers) * colnorm_inv ; softmax ---
    psl = ps_s.tile([B, K], F32, tag="s")
    for i in range(CB):
        nc.tensor.matmul(
            psl[:],
            qnT[:, i * B : (i + 1) * B],
            cen_sb[:, i * K : (i + 1) * K],
            start=(i == 0),
            stop=(i == CB - 1),
        )
    nc.vector.tensor_mul(logit[:], psl[:], sb_bc[:])
    nc.vector.reduce_max(mx[:], logit[:], axis=AX.X)
    nc.vector.tensor_scalar(esh[:], logit[:], mx[:], None, op0=AluOpType.subtract)
    nc.scalar.activation(ex[:], esh[:], AF.Exp, accum_out=se[:])
    nc.vector.reciprocal(rinv[:], se[:])
    nc.vector.tensor_scalar_mul(attn[:], ex[:], rinv[:])
    psa = ps_s.tile([K, B], F32, tag="s")
    nc.tensor.transpose(psa[:], attn[:], ident[:B, :B])
    at_dst = attnT[:]
    if wash:
        at_dst = at_dst.bitcast(F32R)
    nc.vector.tensor_copy(at_dst, psa[:])

    # --- proj_w.T tiles (PE transpose, one-time) ---
    for i in range(CB):  # co chunk
        for j in range(CB):  # ci chunk
            pst = ps_s.tile([128, 128], F32, tag="s")
            nc.tensor.transpose(
                pst[:], pw_sb[:, i * C + j * 128 : i * C + (j + 1) * 128], ident[:]
            )
            pjT_dst = pjT_sb[:, j * C + i * 128 : j * C + (i + 1) * 128]
            nc.vector.tensor_copy(mm7(pjT_dst), pst[:])

    # --- bias broadcast over the 64 n-partitions ---
    psbb = ps7.tile([NL, C], F32, tag="p7")
    nc.tensor.matmul(psbb[:], ones64[:], bias_sb[:])
    nc.vector.tensor_copy(bias64[:], psbb[:])

    # --- step5: tmT[m, b, n] = sum_k T[k, n, m] * attn[b, k] ---
    # The 4 m-chunk matmuls of one n-row share one PSUM bank (one lazily
    # zeroed accumulation group, disjoint 64B slices) -> a single batched
    # PSUM->SBUF copy per n-row instead of four.
    for ch in range(N_TCH):
        if wash:
            tst = tpool.tile([K, T_CHUNK, N], F32, tag="t")
            nc.sync.dma_start(
                out=tst[:], in_=t_d[:, ch * T_CHUNK : (ch + 1) * T_CHUNK, :]
            )
            tch = trp.tile([K, T_CHUNK, N], F32, tag="tr")
            nc.vector.tensor_copy(tch[:].bitcast(F32R), tst[:])
        else:
            tch = tpool.tile([K, T_CHUNK, N], DT, tag="t")
            eng = nc.gpsimd if bf else nc.sync
            eng.dma_start(
                out=tch[:], in_=t_d[:, ch * T_CHUNK : (ch + 1) * T_CHUNK, :]
            )
        for nr in range(T_CHUNK):
            n0 = ch * T_CHUNK + nr
            p5 = ps5.tile([128, MC, B], F32, tag="p5")
            for mc in range(MC):
                nc.tensor.matmul(
                    p5[:, mc, :],
                    mm5(tch[:, nr, mc * 128 : (mc + 1) * 128]),
                    mm5(attnT[:]),
                    start=(mc == 0),
                    stop=(mc == MC - 1),
                )
            tm_dst = tmT_v[:, :, :, n0]
            if wash:
                tm_dst = tm_dst.bitcast(F32R)
            nc.any.tensor_copy(tm_dst, p5[:])

    # --- step6: yT[c, n] per b = sum_m x[b][m, c] * tmT[m, b, n] ---
    for b in range(B):
        for cb in range(CB):
            p6 = ps6.tile([128, NL], F32, tag="p6")
            for mc in range(MC):
                nc.tensor.matmul(
                    p6[:],
                    mm5(
                        x_sb[
                            :,
                            (b * MC + mc) * C
                            + cb * 128 : (b * MC + mc) * C
                            + (cb + 1) * 128,
                        ]
                    ),
                    mm5(tmT_v[:, mc, b, :]),
                    start=(mc == 0),
                    stop=(mc == MC - 1),
                )
            nc.vector.tensor_copy(mm7(yT_v[:, cb, b, :]), p6[:])

    # --- step7: out[n, co] = sum_ci yT[ci, n] * projT[ci, co] + bias ---
    for b in range(B):
        p7 = ps7.tile([NL, C], F32, tag="p7")
        for cb in range(CB):
            nc.tensor.matmul(
                p7[:],
                mm7(yT_v[:, cb, b, :]),
                mm7(pjT_sb[:, cb * C : (cb + 1) * C]),
                start=(cb == 0),
                stop=(cb == CB - 1),
            )
        ot = opool.tile([NL, C], F32, tag="o")
        nc.vector.tensor_add(ot[:], p7[:], bias64[:])
        nc.sync.dma_start(out=out_d[b], in_=ot[:])

    ctx.close()


_CACHE = {}


def build_nc(reps=1, mode="f32rp"):
    key = (reps, mode)
    if key in _CACHE:
        return _CACHE[key]
    nc = bacc.Bacc(
        "TRN2",
        target_bir_lowering=False,
        debug=False,
        enable_asserts=False,
        num_devices=NCORES,
    )
    x_d = nc.dram_tensor("x", [B, N, C], F32, kind="ExternalInput").ap()
    cen_d = nc.dram_tensor("centers", [C, K], F32, kind="ExternalInput").ap()
    pw_d = nc.dram_tensor("proj_w", [C, C], F32, kind="ExternalInput").ap()
    pb_d = nc.dram_tensor("proj_b", [C], F32, kind="ExternalInput").ap()
    t_d = nc.dram_tensor("t_loc", [K, NL, N], F32, kind="ExternalInput").ap()
    out_d = nc.dram_tensor("out_loc", [B, NL, C], F32, kind="ExternalOutput").ap()
    with tile.TileContext(nc) as tc:
        for it in range(reps):
            _emit(tc, x_d, cen_d, pw_d, pb_d, t_d, out_d, it=it, mode=mode)
    nc.compile()
    _CACHE[key] = nc
    return nc


def make_in_maps(x, centers, tran_ms, proj_w, proj_b):
    x = np.ascontiguousarray(x, dtype=np.float32)
    centers = np.ascontiguousarray(centers, dtype=np.float32)
    proj_w = np.ascontiguousarray(proj_w, dtype=np.float32)
    proj_b = np.ascontiguousarray(proj_b, dtype=np.float32)
    in_maps = []
    for core in range(NCORES):
        t_loc = np.ascontiguousarray(
            tran_ms[:, core * NL : (core + 1) * NL, :], dtype=np.float32
        )
        in_maps.append(
            {
                "x": x,
                "centers": centers,
                "proj_w": proj_w,
                "proj_b": proj_b,
                "t_loc": t_loc,
            }
        )
    return in_maps


# Default precision/speed point: bf16 compute hits the per-core HBM
# roofline (~65us/iter vs ~63us floor, measured); fp32-class compute is
# PE-bound ~4x slower on this problem.  Override via KERNEL_MODE env if
# tighter accuracy is needed ("f32rp" ~1.2e-4, "fp32" ~1e-6).
import os  # noqa: E402

MODE = os.environ.get("KERNEL_MODE", "bf16")


def kernel(**inputs):
    from concourse.bass_utils import run_bass_kernel_spmd

    nc = build_nc(mode=MODE)
    in_maps = make_in_maps(
        inputs["x"],
        inputs["centers"],
        inputs["tran_ms"],
        inputs["proj_w"],
        inputs["proj_b"],
    )
    res = run_bass_kernel_spmd(nc, in_maps, list(range(NCORES)))
    out = np.concatenate([res.results[c]["out_loc"] for c in range(NCORES)], axis=1)
    return out.astype(np.float32)
